# revision 2
# baseline (speedup 1.0000x reference)
"""7x7 grayscale dilation (flat SE, zero padding) on Trainium2, 8 NeuronCores.

Strategy (pure data parallel, per sharding hint):
  - shard x (32,3,512,512) by batch: 4 batches -> 12 images of 512x512 per core
  - per image: horizontal 7-window max cascade (shifts 1,2,3) along the free
    dim, PE transpose (via identity matmul) to flip W into partitions,
    vertical cascade along the free dim, PE transpose back, store.
  - all maxes on DVE (the only engine supporting TensorTensor in this stack);
    PSUM->SBUF copies on the scalar (ACT) engine; loads/stores on sync (HWDGE).

se is (7,7) ones in this problem: bias = se-1 = 0 and mask = 1, so the op is
exactly a 7x7 sliding max over the zero-padded input.  A numpy fallback
handles any other se faithfully.
"""
import numpy as np

_CACHE = {}

N_CORES = 8
IMGS = 12  # images per core: 4 batches x 3 channels
H = W = 512


def _build_nc():
    from contextlib import ExitStack
    from concourse import bacc, tile, mybir
    from concourse.masks import make_identity

    F32 = mybir.dt.float32
    MAX = mybir.AluOpType.max

    nc = bacc.Bacc("TRN2", target_bir_lowering=False)
    x_in = nc.dram_tensor("x", [IMGS, H, W], F32, kind="ExternalInput")
    y_out = nc.dram_tensor("y", [IMGS, H, W], F32, kind="ExternalOutput")

    NSLOT = 4

    with tile.TileContext(nc) as tc:
        with ExitStack() as ctx:
            pool = ctx.enter_context(tc.tile_pool(name="p", bufs=1))
            psum = ctx.enter_context(tc.tile_pool(name="ps", bufs=2, space="PSUM"))

            ident = pool.tile([128, 128], F32)
            make_identity(nc, ident[:])

            slots = []
            for s in range(NSLOT):
                b_xt = pool.tile([128, 4, 518], F32, tag=f"xt{s}")
                b_a = pool.tile([128, 4, 517], F32, tag=f"a{s}")
                b_u = pool.tile([128, 4, 517], F32, tag=f"u{s}")
                b_vt = pool.tile([128, 4, 518], F32, tag=f"vt{s}")
                # persistent zero halo columns (never rewritten)
                for t in (b_xt, b_vt):
                    nc.vector.memset(t[:, :, 0:3], 0.0)
                    nc.vector.memset(t[:, :, 515:518], 0.0)
                slots.append((b_xt, b_a, b_u, b_vt))

            for i in range(IMGS):
                xt, b_a, b_u, b_vt = slots[i % NSLOT]

                # load image i: rows r = 128T + p -> xt[p, T, 3+w]
                nc.sync.dma_start(
                    out=xt[:, :, 3:515],
                    in_=x_in[i].rearrange("(t p) w -> p t w", p=128, t=4),
                )

                # horizontal cascade: v[.., q] = max x[.., q-3 .. q+3]
                a = b_a[:, :, 0:517]
                u = b_u[:, :, 0:515]
                v = b_a[:, :, 0:512]
                nc.vector.tensor_tensor(a, xt[:, :, 0:517], xt[:, :, 1:518], op=MAX)
                nc.vector.tensor_tensor(u, b_a[:, :, 0:515], b_a[:, :, 2:517], op=MAX)
                nc.vector.tensor_tensor(v, b_u[:, :, 0:512], b_u[:, :, 3:515], op=MAX)

                # transpose v -> vT  (vT[q, Wb, 3+r] = v[r, 128Wb+q])
                for pair in range(2):  # Wb pairs (0,1), (2,3)
                    P = psum.tile([128, 1024], F32, tag="P")
                    for wp in range(2):
                        Wb = 2 * pair + wp
                        for T in range(4):
                            nc.tensor.matmul(
                                P[:, 512 * wp + 128 * T : 512 * wp + 128 * T + 128],
                                b_a[:, T, 128 * Wb : 128 * Wb + 128],
                                ident[:],
                                is_transpose=True,
                            )
                    nc.scalar.copy(
                        b_vt[:, 2 * pair : 2 * pair + 2, 3:515],
                        P[:].rearrange("p (a b) -> p a b", a=2, b=512),
                    )

                # vertical cascade along free dim of vT
                a2 = b_a[:, :, 0:517]
                u2 = b_u[:, :, 0:515]
                z = b_vt[:, :, 3:515]  # interior only; halo cols preserved
                nc.vector.tensor_tensor(a2, b_vt[:, :, 0:517], b_vt[:, :, 1:518], op=MAX)
                nc.vector.tensor_tensor(u2, b_a[:, :, 0:515], b_a[:, :, 2:517], op=MAX)
                nc.vector.tensor_tensor(z, b_u[:, :, 0:512], b_u[:, :, 3:515], op=MAX)

                # transpose back: ot[p, T, 3+w] = z[w, 128T+p]
                for pair in range(2):  # T pairs
                    P2 = psum.tile([128, 1024], F32, tag="P2")
                    for tp in range(2):
                        T = 2 * pair + tp
                        for Wb in range(4):
                            nc.tensor.matmul(
                                P2[:, 512 * tp + 128 * Wb : 512 * tp + 128 * Wb + 128],
                                b_vt[:, Wb, 3 + 128 * T : 3 + 128 * T + 128],
                                ident[:],
                                is_transpose=True,
                            )
                    nc.scalar.copy(
                        xt[:, 2 * pair : 2 * pair + 2, 3:515],
                        P2[:].rearrange("p (a b) -> p a b", a=2, b=512),
                    )

                # store
                nc.sync.dma_start(
                    out=y_out[i].rearrange("(t p) w -> p t w", p=128, t=4),
                    in_=xt[:, :, 3:515],
                )

    nc.finalize()
    return nc


def _get_nc():
    if "nc" not in _CACHE:
        _CACHE["nc"] = _build_nc()
    return _CACHE["nc"]


def _run_bass(x, trace=False):
    """x: (32,3,512,512) float32 -> (32,3,512,512) float32 via 8 cores."""
    from concourse.bass_utils import run_bass_kernel_spmd

    nc = _get_nc()
    xr = np.ascontiguousarray(x).reshape(N_CORES, IMGS, H, W)
    in_maps = [{"x": xr[k]} for k in range(N_CORES)]
    r = run_bass_kernel_spmd(nc, in_maps, list(range(N_CORES)), trace=trace)
    out = np.stack([r.results[k]["y"] for k in range(N_CORES)], axis=0)
    return out.reshape(32, 3, 512, 512), r


def kernel(x, se):
    x = np.asarray(x, dtype=np.float32)
    se = np.asarray(se, dtype=np.float32)
    if se.shape == (7, 7) and np.all(se == 1.0):
        out, _ = _run_bass(x)
        return out
    # general fallback (never hit for this problem's inputs)
    kh, kw = se.shape
    ph, pw = kh // 2, kw // 2
    bias = se.reshape(-1) - 1.0
    mask = (bias >= 0).astype(x.dtype)
    xp = np.pad(x, ((0, 0), (0, 0), (ph, ph), (pw, pw)))
    out = np.full(x.shape, -np.inf, dtype=x.dtype)
    for i in range(kh * kw):
        r, c = i // kw, i % kw
        win = xp[:, :, r : r + x.shape[2], c : c + x.shape[3]]
        out = np.maximum(out, mask[i] * win + bias[i])
    return out
